# revision 21
# baseline (speedup 1.0000x reference)
"""Sauvola binarization kernel for 8 Trainium2 NeuronCores (data-parallel).

Algorithm (per core, one 1024x1024x3 image):
  gray = RGB dot [0.2989, 0.5870, 0.1140]
  m/m2 = 51x51 reflect-padded box means of gray / gray^2 (via two banded
  fp16 matmul passes on the PE: each pass applies the 51-tap reflect box
  along the partition axis and transposes, so H-pass . W-pass returns to
  the original orientation)
  r = 0.5*(max-min) over ALL images' gray, exchanged via an AllGather of
  per-core (-min, max) pairs (~2x cheaper than AllReduce in latency) and
  folded locally.
  thresh = m*(1 + 0.2*(s/r - 1)),  out = (gray > thresh) as f32, computed
  reciprocal-free as  v1 * (1024*2r) > 64*C^2-scaled v2.

Schedule: phase A streams 8 w-chunks (DMA-paced); chunk 7 is processed as
4 row-pieces so the global min/max - and hence the AllGather launch -
clears a couple of us after the final input byte. Phase B (r-independent
t1/t2/s0/qa16/v1/v2) overlaps the collective; only the final per-chunk
mask STT waits for r. Engine placement respects walrus rules: Pool only
runs tensor_scalar / cross-lane max reduces / the collective; every
PSUM-reading elementwise op and all compares live on DVE; copies on Act.
"""
import numpy as np

import concourse.bass as bass
import concourse.mybir as mybir
import concourse.tile as tile
from concourse.bass_utils import run_bass_kernel_spmd

N_CORES = 8
F = mybir.dt.float32
Hh = mybir.dt.float16
W0, W1, W2 = 0.2989, 0.5870, 0.1140
KS = 0.2
HALF = 25
WINDOWS = [(0, 0, 153), (1, 103, 178), (2, 231, 178), (3, 359, 153), (3, 512, 25),
           (4, 487, 25), (4, 512, 153), (5, 615, 178), (6, 743, 178), (7, 871, 153)]
B0_FIRST, B1_FIRST, B0_LAST, B1_LAST = 0, 4, 5, 9
P0PP = (1.0 - KS) / (2601.0 * W0)
C_BASE = 2.0 * KS / (2601.0 ** 2 * W0) * 128.0
# mask compare: v1 * (1024*2r) > s016*qa16 with s016 = sqrt(64*C_BASE^2 * t2)
S0_SCALE = 64.0 * C_BASE * C_BASE
RSUM_SCALE = 1024.0


def _split_multi_waits(nc):
    """walrus here allows one sync wait per instruction; split extras to NOPs."""
    for func in nc.m.functions:
        for bb in func.blocks:
            insts = bb.instructions
            i = 0
            while i < len(insts):
                inst = insts[i]
                si = inst.sync_info
                if si is None or len(si.on_wait) <= 1:
                    i += 1
                    continue
                waits = list(si.on_wait)
                nops = []
                for w in waits[:-1]:
                    nop = mybir.InstNoOp(
                        name=nc.get_next_instruction_name(),
                        sync_info=mybir.SyncInfo(on_wait=[w], on_update=[]),
                        bass_nofuse=True,
                        engine=inst.engine,
                    )
                    nops.append(nop)
                inst.sync_info = mybir.SyncInfo(
                    on_wait=[waits[-1]], on_update=list(si.on_update)
                )
                for k, nop in enumerate(nops):
                    insts.insert(i + k, nop)
                    nc.register_instruction(nop, overwrite=True)
                i += len(nops) + 1


def _build_band_blocks():
    B = np.zeros((1024, 1024), dtype=np.float32)
    idx = np.arange(1024)
    for d in range(-HALF, HALF + 1):
        t = idx + d
        t = np.where(t < 0, -t, t)
        t = np.where(t > 1023, 2046 - t, t)
        np.add.at(B, (idx, t), 1.0)
    # [128 partitions, 10 windows, 178]: one contiguous 3560B descriptor
    # per partition row.
    blocks = np.zeros((128, len(WINDOWS), 178), dtype=np.float16)
    for k, (i, c0, ncols) in enumerate(WINDOWS):
        blocks[:, k, :ncols] = B[c0:c0 + ncols, 128 * i:128 * (i + 1)].T[:, :]
    return blocks


def _emit_matmuls(nc, ps, band_sb, src_of_a, src_of_b):
    """Both banded matmul groups (gray->pa, g2c->pb) for one chunk."""
    pa = ps.tile([128, 1024], F, tag="A")
    pb = ps.tile([128, 1024], F, tag="B")
    for src_of, pt in ((src_of_a, pa), (src_of_b, pb)):
        for k, (i, c0, ncols) in enumerate(WINDOWS):
            nc.tensor.matmul(
                pt[:, c0:c0 + ncols], src_of(i),
                band_sb[:, k, :ncols],
                start=(k in (B0_FIRST, B1_FIRST)),
                stop=(k in (B0_LAST, B1_LAST)))
    return pa, pb


def _build_nc():
    nc = bass.Bass("TRN2", target_bir_lowering=False, debug=False,
                   num_devices=N_CORES)
    x = nc.dram_tensor("x", [1024, 3072], F, kind="ExternalInput")
    band = nc.dram_tensor("band", [128, len(WINDOWS), 178], Hh,
                          kind="ExternalInput")
    out = nc.dram_tensor("out", [1024, 1024], Hh, kind="ExternalOutput")

    AluOp = mybir.AluOpType
    Act = mybir.ActivationFunctionType
    Ax = mybir.AxisListType

    with tile.TileContext(nc) as tc:
        with (
            tc.tile_pool(name="consts", bufs=1) as consts,
            tc.tile_pool(name="xin", bufs=2) as xin,
            tc.tile_pool(name="work", bufs=2) as work,
            tc.tile_pool(name="keep", bufs=1) as keep,
            tc.tile_pool(name="grayp", bufs=3) as grayp,
            tc.tile_pool(name="tkeep", bufs=8) as tkeep,
            tc.tile_pool(name="vkeep", bufs=8) as vkeep,
            tc.tile_pool(name="maskp", bufs=6) as maskp,
            tc.tile_pool(name="ps", bufs=2, space="PSUM") as ps,
            tc.tile_pool(name="dram", bufs=1, space="DRAM") as dram,
        ):
            xc = x.ap().rearrange("(i p) (j w) -> p i j w", p=128, w=384)

            # chunk-0 input DMA first so compute starts ASAP; band second.
            xj0 = xin.tile([128, 8, 384], F, tag="xj")
            nc.sync.dma_start(xj0[:], xc[:, :, 0, :])
            band_sb = consts.tile([128, len(WINDOWS), 178], Hh)
            nc.sync.dma_start(band_sb[:], band.ap())
            bias_sq = consts.tile([128, 1], F)
            nc.gpsimd.memset(bias_sq[:], -25.5)
            bias_t1 = consts.tile([128, 1], F)
            nc.gpsimd.memset(bias_t1[:], -1300.5)

            u2all = keep.tile([128, 8, 8, 128], F)     # gray / W0, all pixels
            accmin = keep.tile([128, 8, 128], Hh)      # fp16 min folds, ch 0-6
            gmaxs = consts.tile([1, 8], F)             # per-chunk max(u2), Pool
            r7 = keep.tile([128, 2, 4], F)             # chunk-7 piece reduces
            ta_tiles, tb_tiles = [], []

            # ---------------- phase A: w-chunks 0..6 ----------------
            prev_copies = None
            for j in range(7):
                xj = xj0 if j == 0 else xin.tile([128, 8, 384], F, tag="xj")
                if j > 0:
                    nc.sync.dma_start(xj[:], xc[:, :, j, :])
                s3 = xj[:].rearrange("p i (w c) -> p i w c", c=3)

                u1 = work.tile([128, 8, 128], F, tag="u1")
                nc.vector.scalar_tensor_tensor(
                    u1[:], s3[:, :, :, 1], W1 / W0, s3[:, :, :, 0],
                    op0=AluOp.mult, op1=AluOp.add)
                u2 = u2all[:, :, j, :]
                nc.vector.scalar_tensor_tensor(
                    u2, s3[:, :, :, 2], W2 / W0, u1[:],
                    op0=AluOp.mult, op1=AluOp.add)

                gray = grayp.tile([128, 8, 128], Hh, tag="gray")
                nc.gpsimd.tensor_scalar(gray[:], u2, W0, None, op0=AluOp.mult)
                g2c = grayp.tile([128, 8, 128], Hh, tag="g2c")
                nc.scalar.activation(g2c[:], gray[:], Act.Square,
                                     bias=bias_sq[:], scale=51.0)

                # max side on Pool (cross-lane max is legal); min folds on DVE
                nc.gpsimd.tensor_reduce(gmaxs[0:1, j:j + 1], u2, Ax.XYZWC,
                                        AluOp.max)
                if j == 0:
                    nc.vector.tensor_copy(accmin[:], gray[:])
                else:
                    nc.vector.tensor_tensor(accmin[:], accmin[:], gray[:],
                                            op=AluOp.min)

                gf, qf = gray, g2c
                pa, pb = _emit_matmuls(nc, ps, band_sb,
                                       lambda i, t=gf: t[:, i, :],
                                       lambda i, t=qf: t[:, i, :])
                # previous chunk's PSUM->fp16 copies go behind this chunk's
                # g2c on Act so g2c[j] never queues behind a PE wait.
                if prev_copies is not None:
                    for src_ps, dst in prev_copies:
                        nc.scalar.copy(dst[:], src_ps[:])
                ta = tkeep.tile([128, 1024], Hh, tag="ta")
                tb = tkeep.tile([128, 1024], Hh, tag="tb")
                ta_tiles.append(ta)
                tb_tiles.append(tb)
                prev_copies = [(pa, ta), (pb, tb)]

            # chunk-6 copies now: Act drains them while chunk-7 streams in
            for src_ps, dst in prev_copies:
                nc.scalar.copy(dst[:], src_ps[:])
            prev_copies = None

            # ---------------- phase A: w-chunk 7 as 4 row-pieces ----------------
            gray7 = keep.tile([128, 8, 128], Hh)
            g2c7 = keep.tile([128, 8, 128], Hh)
            u2ps = []
            for p in range(4):
                xp = xin.tile([128, 2, 384], F, tag="xp", bufs=4)
                nc.sync.dma_start(xp[:], xc[:, 2 * p:2 * p + 2, 7, :])
                s3 = xp[:].rearrange("p i (w c) -> p i w c", c=3)
                u1 = work.tile([128, 2, 128], F, tag="u1p")
                nc.vector.scalar_tensor_tensor(
                    u1[:], s3[:, :, :, 1], W1 / W0, s3[:, :, :, 0],
                    op0=AluOp.mult, op1=AluOp.add)
                u2 = u2all[:, 2 * p:2 * p + 2, 7, :]
                nc.vector.scalar_tensor_tensor(
                    u2, s3[:, :, :, 2], W2 / W0, u1[:],
                    op0=AluOp.mult, op1=AluOp.add)
                u2ps.append(u2)
                gray_s = gray7[:, 2 * p:2 * p + 2, :]
                nc.gpsimd.tensor_scalar(gray_s, u2, W0, None, op0=AluOp.mult)
                nc.scalar.activation(g2c7[:, 2 * p:2 * p + 2, :], gray_s,
                                     Act.Square, bias=bias_sq[:], scale=51.0)
            with tc.high_priority():
                for p in range(4):
                    # lane-only reduces (legal for min) straight off u2 f32
                    nc.vector.tensor_reduce(r7[:, 0, p:p + 1], u2ps[p], Ax.XY,
                                            AluOp.min)
                    nc.vector.tensor_reduce(r7[:, 1, p:p + 1], u2ps[p], Ax.XY,
                                            AluOp.max)

            # ---------------- r-chain + AllGather launch ----------------
            with tc.high_priority():
                # min side: lane-min then negate, cross-lane via max (legal)
                rmin06 = consts.tile([128, 1], F)
                nc.vector.tensor_reduce(
                    rmin06[:], accmin[:].rearrange("p a b -> p (a b)"),
                    Ax.X, AluOp.min)
                n06 = consts.tile([128, 1], F)
                nc.vector.tensor_scalar(n06[:], rmin06[:], -1.0, None,
                                        op0=AluOp.mult)
                r7m = consts.tile([128, 2], F)
                nc.vector.tensor_reduce(r7m[:, 0:1], r7[:, 0, :], Ax.X,
                                        AluOp.min)
                nc.vector.tensor_reduce(r7m[:, 1:2], r7[:, 1, :], Ax.X,
                                        AluOp.max)
                r7s = consts.tile([128, 2], F)
                nc.vector.tensor_scalar(r7s[:, 0:1], r7m[:, 0:1], -W0, None,
                                        op0=AluOp.mult)
                nc.vector.tensor_scalar(r7s[:, 1:2], r7m[:, 1:2], W0, None,
                                        op0=AluOp.mult)
                negall = consts.tile([128, 1], F)
                nc.vector.tensor_tensor(negall[:], n06[:], r7s[:, 0:1],
                                        op=AluOp.max)
                mm1 = consts.tile([1, 2], F)
                nc.gpsimd.tensor_reduce(mm1[:, 0:1], negall[:], Ax.C,
                                        AluOp.max)
                # max side: chunk 0-6 maxima (u2 units) + chunk-7 pieces
                max7 = consts.tile([1, 1], F)
                nc.gpsimd.tensor_reduce(max7[:], r7s[:, 1:2], Ax.C, AluOp.max)
                gmaxu = consts.tile([1, 1], F)
                nc.vector.tensor_reduce(gmaxu[:], gmaxs[:], Ax.X, AluOp.max)
                gmaxg = consts.tile([1, 1], F)
                nc.vector.tensor_scalar(gmaxg[:], gmaxu[:], W0, None,
                                        op0=AluOp.mult)
                nc.vector.tensor_tensor(mm1[:, 1:2], gmaxg[:], max7[:],
                                        op=AluOp.max)

                mm_in = dram.tile([1, 2], F)
                mm_sh = dram.tile([1, 2 * N_CORES], F, addr_space="Shared")
                nc.sync.dma_start(mm_in[:], mm1[:])
                nc.gpsimd.collective_compute(
                    "AllGather", AluOp.bypass,
                    replica_groups=[list(range(N_CORES))],
                    ins=[mm_in.opt()], outs=[mm_sh.opt()])
                mm_b = consts.tile([128, 2 * N_CORES], F)
                nc.sync.dma_start(mm_b[:],
                                  mm_sh[:].to_broadcast((128, 2 * N_CORES)))

            # chunk-7 matmuls + copies
            pa, pb = _emit_matmuls(nc, ps, band_sb,
                                   lambda i: gray7[:, i, :],
                                   lambda i: g2c7[:, i, :])
            ta = tkeep.tile([128, 1024], Hh, tag="ta")
            tb = tkeep.tile([128, 1024], Hh, tag="tb")
            nc.scalar.copy(ta[:], pa[:])
            nc.scalar.copy(tb[:], pb[:])
            ta_tiles.append(ta)
            tb_tiles.append(tb)

            # ---------------- phase B (r-independent parts) ----------------
            v1_tiles, v2_tiles = [], []
            for m in range(8):
                qa, qb = _emit_matmuls(
                    nc, ps, band_sb,
                    lambda jj: ta_tiles[jj][:, 128 * m:128 * (m + 1)],
                    lambda jj: tb_tiles[jj][:, 128 * m:128 * (m + 1)])
                qa3 = qa[:].rearrange("p (a b) -> p a b", b=128)
                qb3 = qb[:].rearrange("p (a b) -> p a b", b=128)
                t1 = work.tile([128, 8, 128], F, tag="t1")
                nc.scalar.activation(t1[:], qa3, Act.Square, bias=bias_t1[:],
                                     scale=1.0)
                # fp16 copy of qa so v2 runs as a 2x fp16 TT and qa's psum
                # frees early (v1/t1/qa16 are all immediate post-matmul)
                qa16 = work.tile([128, 8, 128], Hh, tag="qa16")
                nc.scalar.copy(qa16[:], qa3)
                v1 = vkeep.tile([128, 8, 128], Hh, tag="v1")
                nc.vector.scalar_tensor_tensor(
                    v1[:], qa3, -P0PP, u2all[:, m, :, :],
                    op0=AluOp.mult, op1=AluOp.add)
                t2 = work.tile([128, 8, 128], F, tag="t2")
                nc.vector.scalar_tensor_tensor(
                    t2[:], t1[:], -1.0, qb3, op0=AluOp.mult, op1=AluOp.add)
                s016 = work.tile([128, 8, 128], Hh, tag="s016")
                nc.scalar.activation(s016[:], t2[:], Act.Sqrt, scale=S0_SCALE)
                v2 = vkeep.tile([128, 8, 128], Hh, tag="v2")
                nc.vector.tensor_tensor(v2[:], s016[:], qa16[:], op=AluOp.mult)
                v1_tiles.append(v1)
                v2_tiles.append(v2)

            # fence: nothing below may be scheduled before the phase-B ops
            tc.no_sync_barrier()

            # r-dependent chain (tiny, DVE): tree-fold gathered (-min, max),
            # rsum6 = 1024*(gmax - gmin)
            mmv = mm_b[:].rearrange("p (a b) -> p a b", b=2)   # [128, 8, 2]
            f1 = consts.tile([128, 4, 2], F)
            nc.vector.tensor_tensor(f1[:], mmv[:, 0:4, :], mmv[:, 4:8, :],
                                    op=AluOp.max)
            f2 = consts.tile([128, 2, 2], F)
            nc.vector.tensor_tensor(f2[:], f1[:, 0:2, :], f1[:, 2:4, :],
                                    op=AluOp.max)
            f3 = consts.tile([128, 2], F)
            nc.vector.tensor_tensor(f3[:], f2[:, 0, :], f2[:, 1, :],
                                    op=AluOp.max)
            rsum = consts.tile([128, 1], F)
            nc.vector.tensor_tensor(rsum[:], f3[:, 0:1], f3[:, 1:2],
                                    op=AluOp.add)
            rsum6 = consts.tile([128, 1], F)
            nc.vector.tensor_scalar(rsum6[:], rsum[:], RSUM_SCALE, None,
                                    op0=AluOp.mult)

            # ---------------- masks: the only r-dependent sweep ----------------
            out_r = out.ap().rearrange("(m p) (a b) -> m p a b", p=128, b=128)
            for m in range(8):
                mask = maskp.tile([128, 8, 128], Hh, tag="mask")
                if m < 6:
                    w = maskp.tile([128, 8, 128], Hh, tag="w", bufs=3)
                    nc.scalar.activation(w[:], v1_tiles[m][:], Act.Copy,
                                         scale=rsum6[:])
                    nc.vector.tensor_tensor(mask[:], w[:], v2_tiles[m][:],
                                            op=AluOp.is_gt)
                else:
                    nc.vector.scalar_tensor_tensor(
                        mask[:], v1_tiles[m][:], rsum6[:], v2_tiles[m][:],
                        op0=AluOp.mult, op1=AluOp.is_gt)
                nc.sync.dma_start(out_r[m], mask[:])

    _split_multi_waits(nc)
    return nc


_CACHE = {}


def _get_nc():
    if "nc" not in _CACHE:
        _CACHE["nc"] = _build_nc()
        _CACHE["band"] = _build_band_blocks()
    return _CACHE["nc"], _CACHE["band"]


def kernel(inputs: np.ndarray) -> np.ndarray:
    nc, band = _get_nc()
    x = np.asarray(inputs, dtype=np.float32)
    in_maps = [
        {"x": np.ascontiguousarray(x[c].reshape(1024, 3072)), "band": band}
        for c in range(N_CORES)
    ]
    res = run_bass_kernel_spmd(nc, in_maps, list(range(N_CORES)))
    masks = [res.results[c]["out"] for c in range(N_CORES)]
    return np.stack(masks)[..., None].astype(np.float32)


# revision 22
# speedup vs baseline: 1.0031x; 1.0031x over previous
"""Sauvola binarization kernel for 8 Trainium2 NeuronCores (data-parallel).

Algorithm (per core, one 1024x1024x3 image):
  gray = RGB dot [0.2989, 0.5870, 0.1140]
  m/m2 = 51x51 reflect-padded box means of gray / gray^2 (via two banded
  fp16 matmul passes on the PE: each pass applies the 51-tap reflect box
  along the partition axis and transposes, so H-pass . W-pass returns to
  the original orientation)
  r = 0.5*(max-min) over ALL images' gray, exchanged via an AllGather of
  per-core (-min, max) pairs (~2x cheaper than AllReduce in latency) and
  folded locally.
  thresh = m*(1 + 0.2*(s/r - 1)),  out = (gray > thresh) as f32, computed
  reciprocal-free as  v1 * (1024*2r) > 64*C^2-scaled v2.

Schedule: phase A streams 8 w-chunks (DMA-paced); chunk 7 is processed as
4 row-pieces so the global min/max - and hence the AllGather launch -
clears a couple of us after the final input byte. Phase B (r-independent
t1/t2/s0/qa16/v1/v2) overlaps the collective; only the final per-chunk
mask STT waits for r. Engine placement respects walrus rules: Pool only
runs tensor_scalar / cross-lane max reduces / the collective; every
PSUM-reading elementwise op and all compares live on DVE; copies on Act.
"""
import numpy as np

import concourse.bass as bass
import concourse.mybir as mybir
import concourse.tile as tile
from concourse.bass_utils import run_bass_kernel_spmd

N_CORES = 8
F = mybir.dt.float32
Hh = mybir.dt.float16
W0, W1, W2 = 0.2989, 0.5870, 0.1140
KS = 0.2
HALF = 25
WINDOWS = [(0, 0, 153), (1, 103, 178), (2, 231, 178), (3, 359, 153), (3, 512, 25),
           (4, 487, 25), (4, 512, 153), (5, 615, 178), (6, 743, 178), (7, 871, 153)]
B0_FIRST, B1_FIRST, B0_LAST, B1_LAST = 0, 4, 5, 9
P0PP = (1.0 - KS) / (2601.0 * W0)
C_BASE = 2.0 * KS / (2601.0 ** 2 * W0) * 128.0
# mask compare: v1 * (1024*2r) > s016*qa16 with s016 = sqrt(64*C_BASE^2 * t2)
S0_SCALE = 64.0 * C_BASE * C_BASE
RSUM_SCALE = 1024.0


def _split_multi_waits(nc):
    """walrus here allows one sync wait per instruction; split extras to NOPs."""
    for func in nc.m.functions:
        for bb in func.blocks:
            insts = bb.instructions
            i = 0
            while i < len(insts):
                inst = insts[i]
                si = inst.sync_info
                if si is None or len(si.on_wait) <= 1:
                    i += 1
                    continue
                waits = list(si.on_wait)
                nops = []
                for w in waits[:-1]:
                    nop = mybir.InstNoOp(
                        name=nc.get_next_instruction_name(),
                        sync_info=mybir.SyncInfo(on_wait=[w], on_update=[]),
                        bass_nofuse=True,
                        engine=inst.engine,
                    )
                    nops.append(nop)
                inst.sync_info = mybir.SyncInfo(
                    on_wait=[waits[-1]], on_update=list(si.on_update)
                )
                for k, nop in enumerate(nops):
                    insts.insert(i + k, nop)
                    nc.register_instruction(nop, overwrite=True)
                i += len(nops) + 1


def _build_band_blocks():
    B = np.zeros((1024, 1024), dtype=np.float32)
    idx = np.arange(1024)
    for d in range(-HALF, HALF + 1):
        t = idx + d
        t = np.where(t < 0, -t, t)
        t = np.where(t > 1023, 2046 - t, t)
        np.add.at(B, (idx, t), 1.0)
    # [128 partitions, 10 windows, 178]: one contiguous 3560B descriptor
    # per partition row.
    blocks = np.zeros((128, len(WINDOWS), 178), dtype=np.float16)
    for k, (i, c0, ncols) in enumerate(WINDOWS):
        blocks[:, k, :ncols] = B[c0:c0 + ncols, 128 * i:128 * (i + 1)].T[:, :]
    return blocks


def _emit_matmuls(nc, ps, band_sb, src_of_a, src_of_b):
    """Both banded matmul groups (gray->pa, g2c->pb) for one chunk."""
    pa = ps.tile([128, 1024], F, tag="A")
    pb = ps.tile([128, 1024], F, tag="B")
    for src_of, pt in ((src_of_a, pa), (src_of_b, pb)):
        for k, (i, c0, ncols) in enumerate(WINDOWS):
            nc.tensor.matmul(
                pt[:, c0:c0 + ncols], src_of(i),
                band_sb[:, k, :ncols],
                start=(k in (B0_FIRST, B1_FIRST)),
                stop=(k in (B0_LAST, B1_LAST)))
    return pa, pb


def _build_nc():
    nc = bass.Bass("TRN2", target_bir_lowering=False, debug=False,
                   num_devices=N_CORES)
    x = nc.dram_tensor("x", [1024, 3072], F, kind="ExternalInput")
    band = nc.dram_tensor("band", [128, len(WINDOWS), 178], Hh,
                          kind="ExternalInput")
    out = nc.dram_tensor("out", [1024, 1024], Hh, kind="ExternalOutput")

    AluOp = mybir.AluOpType
    Act = mybir.ActivationFunctionType
    Ax = mybir.AxisListType

    with tile.TileContext(nc) as tc:
        with (
            tc.tile_pool(name="consts", bufs=1) as consts,
            tc.tile_pool(name="xin", bufs=2) as xin,
            tc.tile_pool(name="work", bufs=2) as work,
            tc.tile_pool(name="keep", bufs=1) as keep,
            tc.tile_pool(name="grayp", bufs=3) as grayp,
            tc.tile_pool(name="tkeep", bufs=8) as tkeep,
            tc.tile_pool(name="vkeep", bufs=8) as vkeep,
            tc.tile_pool(name="maskp", bufs=8) as maskp,
            tc.tile_pool(name="ps", bufs=2, space="PSUM") as ps,
            tc.tile_pool(name="dram", bufs=1, space="DRAM") as dram,
        ):
            xc = x.ap().rearrange("(i p) (j w) -> p i j w", p=128, w=384)

            # chunk-0 input DMA first so compute starts ASAP; band second.
            xj0 = xin.tile([128, 8, 384], F, tag="xj")
            nc.sync.dma_start(xj0[:], xc[:, :, 0, :])
            band_sb = consts.tile([128, len(WINDOWS), 178], Hh)
            nc.sync.dma_start(band_sb[:], band.ap())
            bias_sq = consts.tile([128, 1], F)
            nc.gpsimd.memset(bias_sq[:], -25.5)
            bias_t1 = consts.tile([128, 1], F)
            nc.gpsimd.memset(bias_t1[:], -1300.5)

            u2all = keep.tile([128, 8, 8, 128], F)     # gray / W0, all pixels
            accmin = keep.tile([128, 8, 128], Hh)      # fp16 min folds, ch 0-6
            gmaxs = consts.tile([1, 8], F)             # per-chunk max(u2), Pool
            r7 = keep.tile([128, 2, 4], F)             # chunk-7 piece reduces
            ta_tiles, tb_tiles = [], []

            # ---------------- phase A: w-chunks 0..6 ----------------
            prev_copies = None
            for j in range(7):
                xj = xj0 if j == 0 else xin.tile([128, 8, 384], F, tag="xj")
                if j > 0:
                    nc.sync.dma_start(xj[:], xc[:, :, j, :])
                s3 = xj[:].rearrange("p i (w c) -> p i w c", c=3)

                u1 = work.tile([128, 8, 128], F, tag="u1")
                nc.vector.scalar_tensor_tensor(
                    u1[:], s3[:, :, :, 1], W1 / W0, s3[:, :, :, 0],
                    op0=AluOp.mult, op1=AluOp.add)
                u2 = u2all[:, :, j, :]
                nc.vector.scalar_tensor_tensor(
                    u2, s3[:, :, :, 2], W2 / W0, u1[:],
                    op0=AluOp.mult, op1=AluOp.add)

                gray = grayp.tile([128, 8, 128], Hh, tag="gray")
                nc.gpsimd.tensor_scalar(gray[:], u2, W0, None, op0=AluOp.mult)
                g2c = grayp.tile([128, 8, 128], Hh, tag="g2c")
                nc.scalar.activation(g2c[:], gray[:], Act.Square,
                                     bias=bias_sq[:], scale=51.0)

                # max side on Pool (cross-lane max is legal); min folds on DVE
                nc.gpsimd.tensor_reduce(gmaxs[0:1, j:j + 1], u2, Ax.XYZWC,
                                        AluOp.max)
                if j == 0:
                    nc.vector.tensor_copy(accmin[:], gray[:])
                else:
                    nc.vector.tensor_tensor(accmin[:], accmin[:], gray[:],
                                            op=AluOp.min)

                gf, qf = gray, g2c
                pa, pb = _emit_matmuls(nc, ps, band_sb,
                                       lambda i, t=gf: t[:, i, :],
                                       lambda i, t=qf: t[:, i, :])
                # previous chunk's PSUM->fp16 copies go behind this chunk's
                # g2c on Act so g2c[j] never queues behind a PE wait.
                if prev_copies is not None:
                    for src_ps, dst in prev_copies:
                        nc.scalar.copy(dst[:], src_ps[:])
                ta = tkeep.tile([128, 1024], Hh, tag="ta")
                tb = tkeep.tile([128, 1024], Hh, tag="tb")
                ta_tiles.append(ta)
                tb_tiles.append(tb)
                prev_copies = [(pa, ta), (pb, tb)]

            # chunk-6 copies now: Act drains them while chunk-7 streams in
            for src_ps, dst in prev_copies:
                nc.scalar.copy(dst[:], src_ps[:])
            prev_copies = None

            # ---------------- phase A: w-chunk 7 as 4 row-pieces ----------------
            gray7 = keep.tile([128, 8, 128], Hh)
            g2c7 = keep.tile([128, 8, 128], Hh)
            u2ps = []
            for p in range(4):
                xp = xin.tile([128, 2, 384], F, tag="xp", bufs=4)
                nc.sync.dma_start(xp[:], xc[:, 2 * p:2 * p + 2, 7, :])
                s3 = xp[:].rearrange("p i (w c) -> p i w c", c=3)
                u1 = work.tile([128, 2, 128], F, tag="u1p")
                nc.vector.scalar_tensor_tensor(
                    u1[:], s3[:, :, :, 1], W1 / W0, s3[:, :, :, 0],
                    op0=AluOp.mult, op1=AluOp.add)
                u2 = u2all[:, 2 * p:2 * p + 2, 7, :]
                nc.vector.scalar_tensor_tensor(
                    u2, s3[:, :, :, 2], W2 / W0, u1[:],
                    op0=AluOp.mult, op1=AluOp.add)
                u2ps.append(u2)
                gray_s = gray7[:, 2 * p:2 * p + 2, :]
                nc.gpsimd.tensor_scalar(gray_s, u2, W0, None, op0=AluOp.mult)
                nc.scalar.activation(g2c7[:, 2 * p:2 * p + 2, :], gray_s,
                                     Act.Square, bias=bias_sq[:], scale=51.0)
            with tc.high_priority():
                for p in range(4):
                    # lane-only reduces (legal for min) straight off u2 f32
                    nc.vector.tensor_reduce(r7[:, 0, p:p + 1], u2ps[p], Ax.XY,
                                            AluOp.min)
                    nc.vector.tensor_reduce(r7[:, 1, p:p + 1], u2ps[p], Ax.XY,
                                            AluOp.max)

            # ---------------- r-chain + AllGather launch ----------------
            with tc.high_priority():
                # min side: lane-min then negate, cross-lane via max (legal)
                rmin06 = consts.tile([128, 1], F)
                nc.vector.tensor_reduce(
                    rmin06[:], accmin[:].rearrange("p a b -> p (a b)"),
                    Ax.X, AluOp.min)
                n06 = consts.tile([128, 1], F)
                nc.vector.tensor_scalar(n06[:], rmin06[:], -1.0, None,
                                        op0=AluOp.mult)
                r7m = consts.tile([128, 2], F)
                nc.vector.tensor_reduce(r7m[:, 0:1], r7[:, 0, :], Ax.X,
                                        AluOp.min)
                nc.vector.tensor_reduce(r7m[:, 1:2], r7[:, 1, :], Ax.X,
                                        AluOp.max)
                r7s = consts.tile([128, 2], F)
                nc.vector.tensor_scalar(r7s[:, 0:1], r7m[:, 0:1], -W0, None,
                                        op0=AluOp.mult)
                nc.vector.tensor_scalar(r7s[:, 1:2], r7m[:, 1:2], W0, None,
                                        op0=AluOp.mult)
                negall = consts.tile([128, 1], F)
                nc.vector.tensor_tensor(negall[:], n06[:], r7s[:, 0:1],
                                        op=AluOp.max)
                mm1 = consts.tile([1, 2], F)
                nc.gpsimd.tensor_reduce(mm1[:, 0:1], negall[:], Ax.C,
                                        AluOp.max)
                # max side: chunk 0-6 maxima (u2 units) + chunk-7 pieces
                max7 = consts.tile([1, 1], F)
                nc.gpsimd.tensor_reduce(max7[:], r7s[:, 1:2], Ax.C, AluOp.max)
                gmaxu = consts.tile([1, 1], F)
                nc.vector.tensor_reduce(gmaxu[:], gmaxs[:], Ax.X, AluOp.max)
                gmaxg = consts.tile([1, 1], F)
                nc.vector.tensor_scalar(gmaxg[:], gmaxu[:], W0, None,
                                        op0=AluOp.mult)
                nc.vector.tensor_tensor(mm1[:, 1:2], gmaxg[:], max7[:],
                                        op=AluOp.max)

                mm_in = dram.tile([1, 2], F)
                mm_sh = dram.tile([1, 2 * N_CORES], F, addr_space="Shared")
                nc.sync.dma_start(mm_in[:], mm1[:])
                nc.gpsimd.collective_compute(
                    "AllGather", AluOp.bypass,
                    replica_groups=[list(range(N_CORES))],
                    ins=[mm_in.opt()], outs=[mm_sh.opt()])
                mm_b = consts.tile([128, 2 * N_CORES], F)
                nc.sync.dma_start(mm_b[:],
                                  mm_sh[:].to_broadcast((128, 2 * N_CORES)))

            # chunk-7 matmuls + copies
            pa, pb = _emit_matmuls(nc, ps, band_sb,
                                   lambda i: gray7[:, i, :],
                                   lambda i: g2c7[:, i, :])
            ta = tkeep.tile([128, 1024], Hh, tag="ta")
            tb = tkeep.tile([128, 1024], Hh, tag="tb")
            nc.scalar.copy(ta[:], pa[:])
            nc.scalar.copy(tb[:], pb[:])
            ta_tiles.append(ta)
            tb_tiles.append(tb)

            # ---------------- phase B (r-independent parts) ----------------
            v1_tiles, v2_tiles = [], []
            for m in range(8):
                qa, qb = _emit_matmuls(
                    nc, ps, band_sb,
                    lambda jj: ta_tiles[jj][:, 128 * m:128 * (m + 1)],
                    lambda jj: tb_tiles[jj][:, 128 * m:128 * (m + 1)])
                qa3 = qa[:].rearrange("p (a b) -> p a b", b=128)
                qb3 = qb[:].rearrange("p (a b) -> p a b", b=128)
                t1 = work.tile([128, 8, 128], F, tag="t1")
                nc.scalar.activation(t1[:], qa3, Act.Square, bias=bias_t1[:],
                                     scale=1.0)
                # fp16 copy of qa so v2 runs as a 2x fp16 TT and qa's psum
                # frees early (v1/t1/qa16 are all immediate post-matmul)
                qa16 = work.tile([128, 8, 128], Hh, tag="qa16")
                nc.scalar.copy(qa16[:], qa3)
                v1 = vkeep.tile([128, 8, 128], Hh, tag="v1")
                nc.vector.scalar_tensor_tensor(
                    v1[:], qa3, -P0PP, u2all[:, m, :, :],
                    op0=AluOp.mult, op1=AluOp.add)
                t2 = work.tile([128, 8, 128], F, tag="t2")
                nc.vector.scalar_tensor_tensor(
                    t2[:], t1[:], -1.0, qb3, op0=AluOp.mult, op1=AluOp.add)
                s016 = work.tile([128, 8, 128], Hh, tag="s016")
                nc.scalar.activation(s016[:], t2[:], Act.Sqrt, scale=S0_SCALE)
                v2 = vkeep.tile([128, 8, 128], Hh, tag="v2")
                nc.vector.tensor_tensor(v2[:], s016[:], qa16[:], op=AluOp.mult)
                v1_tiles.append(v1)
                v2_tiles.append(v2)

            # fence: nothing below may be scheduled before the phase-B ops
            tc.no_sync_barrier()

            # r-dependent chain (tiny, DVE): tree-fold gathered (-min, max),
            # rsum6 = 1024*(gmax - gmin)
            mmv = mm_b[:].rearrange("p (a b) -> p a b", b=2)   # [128, 8, 2]
            f1 = consts.tile([128, 4, 2], F)
            nc.vector.tensor_tensor(f1[:], mmv[:, 0:4, :], mmv[:, 4:8, :],
                                    op=AluOp.max)
            f2 = consts.tile([128, 2, 2], F)
            nc.vector.tensor_tensor(f2[:], f1[:, 0:2, :], f1[:, 2:4, :],
                                    op=AluOp.max)
            f3 = consts.tile([128, 2], F)
            nc.vector.tensor_tensor(f3[:], f2[:, 0, :], f2[:, 1, :],
                                    op=AluOp.max)
            rsum = consts.tile([128, 1], F)
            nc.vector.tensor_tensor(rsum[:], f3[:, 0:1], f3[:, 1:2],
                                    op=AluOp.add)
            rsum6 = consts.tile([128, 1], F)
            nc.vector.tensor_scalar(rsum6[:], rsum[:], RSUM_SCALE, None,
                                    op0=AluOp.mult)

            # ---------------- masks: the only r-dependent sweep ----------------
            out_r = out.ap().rearrange("(m p) (a b) -> m p a b", p=128, b=128)
            for m in range(8):
                mask = maskp.tile([128, 8, 128], Hh, tag="mask")
                nc.vector.scalar_tensor_tensor(
                    mask[:], v1_tiles[m][:], rsum6[:], v2_tiles[m][:],
                    op0=AluOp.mult, op1=AluOp.is_gt)
                nc.sync.dma_start(out_r[m], mask[:])

    _split_multi_waits(nc)
    return nc


_CACHE = {}


def _get_nc():
    if "nc" not in _CACHE:
        _CACHE["nc"] = _build_nc()
        _CACHE["band"] = _build_band_blocks()
    return _CACHE["nc"], _CACHE["band"]


def kernel(inputs: np.ndarray) -> np.ndarray:
    nc, band = _get_nc()
    x = np.asarray(inputs, dtype=np.float32)
    in_maps = [
        {"x": np.ascontiguousarray(x[c].reshape(1024, 3072)), "band": band}
        for c in range(N_CORES)
    ]
    res = run_bass_kernel_spmd(nc, in_maps, list(range(N_CORES)))
    masks = [res.results[c]["out"] for c in range(N_CORES)]
    return np.stack(masks)[..., None].astype(np.float32)


# revision 23
# speedup vs baseline: 1.0093x; 1.0062x over previous
"""Sauvola binarization kernel for 8 Trainium2 NeuronCores (data-parallel).

Algorithm (per core, one 1024x1024x3 image):
  gray = RGB dot [0.2989, 0.5870, 0.1140]
  m/m2 = 51x51 reflect-padded box means of gray / gray^2 (via two banded
  fp16 matmul passes on the PE: each pass applies the 51-tap reflect box
  along the partition axis and transposes, so H-pass . W-pass returns to
  the original orientation)
  r = 0.5*(max-min) over ALL images' gray, exchanged via an AllGather of
  per-core (-min, max) pairs (~2x cheaper than AllReduce in latency) and
  folded locally.
  thresh = m*(1 + 0.2*(s/r - 1)),  out = (gray > thresh) as f32, computed
  reciprocal-free as  v1 * (1024*2r) > 64*C^2-scaled v2.

Schedule: phase A streams 8 w-chunks (DMA-paced); chunk 7 is processed as
4 row-pieces so the global min/max - and hence the AllGather launch -
clears a couple of us after the final input byte. Phase B (r-independent
t1/t2/s0/qa16/v1/v2) overlaps the collective; only the final per-chunk
mask STT waits for r. Engine placement respects walrus rules: Pool only
runs tensor_scalar / cross-lane max reduces / the collective; every
PSUM-reading elementwise op and all compares live on DVE; copies on Act.
"""
import numpy as np

import concourse.bass as bass
import concourse.mybir as mybir
import concourse.tile as tile
from concourse.bass_utils import run_bass_kernel_spmd

N_CORES = 8
F = mybir.dt.float32
Hh = mybir.dt.float16
W0, W1, W2 = 0.2989, 0.5870, 0.1140
KS = 0.2
HALF = 25
WINDOWS = [(0, 0, 153), (1, 103, 178), (2, 231, 178), (3, 359, 153), (3, 512, 25),
           (4, 487, 25), (4, 512, 153), (5, 615, 178), (6, 743, 178), (7, 871, 153)]
B0_FIRST, B1_FIRST, B0_LAST, B1_LAST = 0, 4, 5, 9
P0PP = (1.0 - KS) / (2601.0 * W0)
C_BASE = 2.0 * KS / (2601.0 ** 2 * W0) * 128.0
# mask compare: v1 * (1024*2r) > s016*qa16 with s016 = sqrt(64*C_BASE^2 * t2)
S0_SCALE = 64.0 * C_BASE * C_BASE
RSUM_SCALE = 1024.0


def _split_multi_waits(nc):
    """walrus here allows one sync wait per instruction; split extras to NOPs."""
    for func in nc.m.functions:
        for bb in func.blocks:
            insts = bb.instructions
            i = 0
            while i < len(insts):
                inst = insts[i]
                si = inst.sync_info
                if si is None or len(si.on_wait) <= 1:
                    i += 1
                    continue
                waits = list(si.on_wait)
                nops = []
                for w in waits[:-1]:
                    nop = mybir.InstNoOp(
                        name=nc.get_next_instruction_name(),
                        sync_info=mybir.SyncInfo(on_wait=[w], on_update=[]),
                        bass_nofuse=True,
                        engine=inst.engine,
                    )
                    nops.append(nop)
                inst.sync_info = mybir.SyncInfo(
                    on_wait=[waits[-1]], on_update=list(si.on_update)
                )
                for k, nop in enumerate(nops):
                    insts.insert(i + k, nop)
                    nc.register_instruction(nop, overwrite=True)
                i += len(nops) + 1


def _build_band_blocks():
    B = np.zeros((1024, 1024), dtype=np.float32)
    idx = np.arange(1024)
    for d in range(-HALF, HALF + 1):
        t = idx + d
        t = np.where(t < 0, -t, t)
        t = np.where(t > 1023, 2046 - t, t)
        np.add.at(B, (idx, t), 1.0)
    # [128 partitions, 10 windows, 178]: one contiguous 3560B descriptor
    # per partition row.
    blocks = np.zeros((128, len(WINDOWS), 178), dtype=np.float16)
    for k, (i, c0, ncols) in enumerate(WINDOWS):
        blocks[:, k, :ncols] = B[c0:c0 + ncols, 128 * i:128 * (i + 1)].T[:, :]
    return blocks


def _emit_matmuls(nc, ps, band_sb, src_of_a, src_of_b):
    """Both banded matmul groups (gray->pa, g2c->pb) for one chunk."""
    pa = ps.tile([128, 1024], F, tag="A")
    pb = ps.tile([128, 1024], F, tag="B")
    for src_of, pt in ((src_of_a, pa), (src_of_b, pb)):
        for k, (i, c0, ncols) in enumerate(WINDOWS):
            nc.tensor.matmul(
                pt[:, c0:c0 + ncols], src_of(i),
                band_sb[:, k, :ncols],
                start=(k in (B0_FIRST, B1_FIRST)),
                stop=(k in (B0_LAST, B1_LAST)))
    return pa, pb


def _build_nc():
    nc = bass.Bass("TRN2", target_bir_lowering=False, debug=False,
                   num_devices=N_CORES)
    x = nc.dram_tensor("x", [1024, 3072], F, kind="ExternalInput")
    band = nc.dram_tensor("band", [128, len(WINDOWS), 178], Hh,
                          kind="ExternalInput")
    out = nc.dram_tensor("out", [1024, 1024], Hh, kind="ExternalOutput")

    AluOp = mybir.AluOpType
    Act = mybir.ActivationFunctionType
    Ax = mybir.AxisListType

    with tile.TileContext(nc) as tc:
        with (
            tc.tile_pool(name="consts", bufs=1) as consts,
            tc.tile_pool(name="xin", bufs=2) as xin,
            tc.tile_pool(name="work", bufs=2) as work,
            tc.tile_pool(name="keep", bufs=1) as keep,
            tc.tile_pool(name="grayp", bufs=3) as grayp,
            tc.tile_pool(name="tkeep", bufs=8) as tkeep,
            tc.tile_pool(name="vkeep", bufs=8) as vkeep,
            tc.tile_pool(name="maskp", bufs=8) as maskp,
            tc.tile_pool(name="ps", bufs=2, space="PSUM") as ps,
            tc.tile_pool(name="dram", bufs=1, space="DRAM") as dram,
        ):
            xc = x.ap().rearrange("(i p) (j w) -> p i j w", p=128, w=384)

            # chunk-0 input DMA first so compute starts ASAP; band second.
            xj0 = xin.tile([128, 8, 384], F, tag="xj")
            nc.sync.dma_start(xj0[:], xc[:, :, 0, :])
            band_sb = consts.tile([128, len(WINDOWS), 178], Hh)
            nc.sync.dma_start(band_sb[:], band.ap())
            bias_sq = consts.tile([128, 1], F)
            nc.gpsimd.memset(bias_sq[:], -25.5)
            bias_t1 = consts.tile([128, 1], F)
            nc.gpsimd.memset(bias_t1[:], -1300.5)

            u2all = keep.tile([128, 8, 8, 128], F)     # gray / W0, all pixels
            accmin = keep.tile([128, 8, 128], Hh)      # fp16 min folds, ch 0-6
            gmaxs = consts.tile([1, 8], F)             # per-chunk max(u2), Pool
            r7 = keep.tile([128, 2, 4], F)             # chunk-7 piece reduces
            ta_tiles, tb_tiles = [], []

            # ---------------- phase A: w-chunks 0..6 ----------------
            prev_copies = None
            for j in range(7):
                xj = xj0 if j == 0 else xin.tile([128, 8, 384], F, tag="xj")
                if j > 0:
                    nc.sync.dma_start(xj[:], xc[:, :, j, :])
                s3 = xj[:].rearrange("p i (w c) -> p i w c", c=3)

                u1 = work.tile([128, 8, 128], F, tag="u1")
                nc.vector.scalar_tensor_tensor(
                    u1[:], s3[:, :, :, 1], W1 / W0, s3[:, :, :, 0],
                    op0=AluOp.mult, op1=AluOp.add)
                u2 = u2all[:, :, j, :]
                nc.vector.scalar_tensor_tensor(
                    u2, s3[:, :, :, 2], W2 / W0, u1[:],
                    op0=AluOp.mult, op1=AluOp.add)

                gray = grayp.tile([128, 8, 128], Hh, tag="gray")
                nc.gpsimd.tensor_scalar(gray[:], u2, W0, None, op0=AluOp.mult)
                g2c = grayp.tile([128, 8, 128], Hh, tag="g2c")
                nc.scalar.activation(g2c[:], gray[:], Act.Square,
                                     bias=bias_sq[:], scale=51.0)

                # max side on Pool (cross-lane max is legal); min folds on DVE
                nc.gpsimd.tensor_reduce(gmaxs[0:1, j:j + 1], u2, Ax.XYZWC,
                                        AluOp.max)
                if j == 0:
                    nc.vector.tensor_copy(accmin[:], gray[:])
                else:
                    nc.vector.tensor_tensor(accmin[:], accmin[:], gray[:],
                                            op=AluOp.min)

                gf, qf = gray, g2c
                pa, pb = _emit_matmuls(nc, ps, band_sb,
                                       lambda i, t=gf: t[:, i, :],
                                       lambda i, t=qf: t[:, i, :])
                # previous chunk's PSUM->fp16 copies go behind this chunk's
                # g2c on Act so g2c[j] never queues behind a PE wait.
                if prev_copies is not None:
                    for src_ps, dst in prev_copies:
                        nc.scalar.copy(dst[:], src_ps[:])
                ta = tkeep.tile([128, 1024], Hh, tag="ta")
                tb = tkeep.tile([128, 1024], Hh, tag="tb")
                ta_tiles.append(ta)
                tb_tiles.append(tb)
                prev_copies = [(pa, ta), (pb, tb)]

            # ---------------- phase A: w-chunk 7 as 4 row-pieces ----------------
            gray7 = keep.tile([128, 8, 128], Hh)
            g2c7 = keep.tile([128, 8, 128], Hh)
            u2ps = []
            for p in range(4):
                xp = xin.tile([128, 2, 384], F, tag="xp", bufs=4)
                nc.sync.dma_start(xp[:], xc[:, 2 * p:2 * p + 2, 7, :])
                s3 = xp[:].rearrange("p i (w c) -> p i w c", c=3)
                u1 = work.tile([128, 2, 128], F, tag="u1p")
                nc.vector.scalar_tensor_tensor(
                    u1[:], s3[:, :, :, 1], W1 / W0, s3[:, :, :, 0],
                    op0=AluOp.mult, op1=AluOp.add)
                u2 = u2all[:, 2 * p:2 * p + 2, 7, :]
                nc.vector.scalar_tensor_tensor(
                    u2, s3[:, :, :, 2], W2 / W0, u1[:],
                    op0=AluOp.mult, op1=AluOp.add)
                u2ps.append(u2)
                gray_s = gray7[:, 2 * p:2 * p + 2, :]
                nc.gpsimd.tensor_scalar(gray_s, u2, W0, None, op0=AluOp.mult)
                nc.scalar.activation(g2c7[:, 2 * p:2 * p + 2, :], gray_s,
                                     Act.Square, bias=bias_sq[:], scale=51.0)
            with tc.high_priority():
                for p in range(4):
                    # lane-only reduces (legal for min) straight off u2 f32
                    nc.vector.tensor_reduce(r7[:, 0, p:p + 1], u2ps[p], Ax.XY,
                                            AluOp.min)
                    nc.vector.tensor_reduce(r7[:, 1, p:p + 1], u2ps[p], Ax.XY,
                                            AluOp.max)

            # ---------------- r-chain + AllGather launch ----------------
            with tc.high_priority():
                # min side: lane-min then negate, cross-lane via max (legal)
                rmin06 = consts.tile([128, 1], F)
                nc.vector.tensor_reduce(
                    rmin06[:], accmin[:].rearrange("p a b -> p (a b)"),
                    Ax.X, AluOp.min)
                n06 = consts.tile([128, 1], F)
                nc.vector.tensor_scalar(n06[:], rmin06[:], -1.0, None,
                                        op0=AluOp.mult)
                r7m = consts.tile([128, 2], F)
                nc.vector.tensor_reduce(r7m[:, 0:1], r7[:, 0, :], Ax.X,
                                        AluOp.min)
                nc.vector.tensor_reduce(r7m[:, 1:2], r7[:, 1, :], Ax.X,
                                        AluOp.max)
                r7s = consts.tile([128, 2], F)
                nc.vector.tensor_scalar(r7s[:, 0:1], r7m[:, 0:1], -W0, None,
                                        op0=AluOp.mult)
                nc.vector.tensor_scalar(r7s[:, 1:2], r7m[:, 1:2], W0, None,
                                        op0=AluOp.mult)
                negall = consts.tile([128, 1], F)
                nc.vector.tensor_tensor(negall[:], n06[:], r7s[:, 0:1],
                                        op=AluOp.max)
                mm1 = consts.tile([1, 2], F)
                nc.gpsimd.tensor_reduce(mm1[:, 0:1], negall[:], Ax.C,
                                        AluOp.max)
                # max side: chunk 0-6 maxima (u2 units) + chunk-7 pieces
                max7 = consts.tile([1, 1], F)
                nc.gpsimd.tensor_reduce(max7[:], r7s[:, 1:2], Ax.C, AluOp.max)
                gmaxu = consts.tile([1, 1], F)
                nc.vector.tensor_reduce(gmaxu[:], gmaxs[:], Ax.X, AluOp.max)
                gmaxg = consts.tile([1, 1], F)
                nc.vector.tensor_scalar(gmaxg[:], gmaxu[:], W0, None,
                                        op0=AluOp.mult)
                nc.vector.tensor_tensor(mm1[:, 1:2], gmaxg[:], max7[:],
                                        op=AluOp.max)

                mm_in = dram.tile([1, 2], F)
                mm_sh = dram.tile([1, 2 * N_CORES], F, addr_space="Shared")
                nc.sync.dma_start(mm_in[:], mm1[:])
                nc.gpsimd.collective_compute(
                    "AllGather", AluOp.bypass,
                    replica_groups=[list(range(N_CORES))],
                    ins=[mm_in.opt()], outs=[mm_sh.opt()])
                mm_b = consts.tile([128, 2 * N_CORES], F)
                nc.sync.dma_start(mm_b[:],
                                  mm_sh[:].to_broadcast((128, 2 * N_CORES)))

            # chunk-7 matmuls + copies
            pa, pb = _emit_matmuls(nc, ps, band_sb,
                                   lambda i: gray7[:, i, :],
                                   lambda i: g2c7[:, i, :])
            ta = tkeep.tile([128, 1024], Hh, tag="ta")
            tb = tkeep.tile([128, 1024], Hh, tag="tb")
            for src_ps, dst in prev_copies:
                nc.scalar.copy(dst[:], src_ps[:])
            nc.scalar.copy(ta[:], pa[:])
            nc.scalar.copy(tb[:], pb[:])
            ta_tiles.append(ta)
            tb_tiles.append(tb)

            # ---------------- phase B (r-independent parts) ----------------
            v1_tiles, v2_tiles = [], []
            for m in range(8):
                qa, qb = _emit_matmuls(
                    nc, ps, band_sb,
                    lambda jj: ta_tiles[jj][:, 128 * m:128 * (m + 1)],
                    lambda jj: tb_tiles[jj][:, 128 * m:128 * (m + 1)])
                qa3 = qa[:].rearrange("p (a b) -> p a b", b=128)
                qb3 = qb[:].rearrange("p (a b) -> p a b", b=128)
                t1 = work.tile([128, 8, 128], F, tag="t1")
                nc.scalar.activation(t1[:], qa3, Act.Square, bias=bias_t1[:],
                                     scale=1.0)
                # fp16 copy of qa so v2 runs as a 2x fp16 TT and qa's psum
                # frees early (v1/t1/qa16 are all immediate post-matmul)
                qa16 = work.tile([128, 8, 128], Hh, tag="qa16")
                nc.scalar.copy(qa16[:], qa3)
                v1 = vkeep.tile([128, 8, 128], Hh, tag="v1")
                nc.vector.scalar_tensor_tensor(
                    v1[:], qa3, -P0PP, u2all[:, m, :, :],
                    op0=AluOp.mult, op1=AluOp.add)
                t2 = work.tile([128, 8, 128], F, tag="t2")
                nc.vector.scalar_tensor_tensor(
                    t2[:], t1[:], -1.0, qb3, op0=AluOp.mult, op1=AluOp.add)
                s016 = work.tile([128, 8, 128], Hh, tag="s016")
                nc.scalar.activation(s016[:], t2[:], Act.Sqrt, scale=S0_SCALE)
                v2 = vkeep.tile([128, 8, 128], Hh, tag="v2")
                nc.vector.tensor_tensor(v2[:], s016[:], qa16[:], op=AluOp.mult)
                v1_tiles.append(v1)
                v2_tiles.append(v2)

            # fence: nothing below may be scheduled before the phase-B ops
            tc.no_sync_barrier()

            # r-dependent chain (tiny, DVE): tree-fold gathered (-min, max),
            # rsum6 = 1024*(gmax - gmin)
            mmv = mm_b[:].rearrange("p (a b) -> p a b", b=2)   # [128, 8, 2]
            f1 = consts.tile([128, 4, 2], F)
            nc.vector.tensor_tensor(f1[:], mmv[:, 0:4, :], mmv[:, 4:8, :],
                                    op=AluOp.max)
            f2 = consts.tile([128, 2, 2], F)
            nc.vector.tensor_tensor(f2[:], f1[:, 0:2, :], f1[:, 2:4, :],
                                    op=AluOp.max)
            f3 = consts.tile([128, 2], F)
            nc.vector.tensor_tensor(f3[:], f2[:, 0, :], f2[:, 1, :],
                                    op=AluOp.max)
            rsum = consts.tile([128, 1], F)
            nc.vector.tensor_tensor(rsum[:], f3[:, 0:1], f3[:, 1:2],
                                    op=AluOp.add)
            rsum6 = consts.tile([128, 1], F)
            nc.vector.tensor_scalar(rsum6[:], rsum[:], RSUM_SCALE, None,
                                    op0=AluOp.mult)

            # ---------------- masks: the only r-dependent sweep ----------------
            out_r = out.ap().rearrange("(m p) (a b) -> m p a b", p=128, b=128)
            for m in range(8):
                mask = maskp.tile([128, 8, 128], Hh, tag="mask")
                nc.vector.scalar_tensor_tensor(
                    mask[:], v1_tiles[m][:], rsum6[:], v2_tiles[m][:],
                    op0=AluOp.mult, op1=AluOp.is_gt)
                nc.sync.dma_start(out_r[m], mask[:])

    _split_multi_waits(nc)
    return nc


_CACHE = {}


def _get_nc():
    if "nc" not in _CACHE:
        _CACHE["nc"] = _build_nc()
        _CACHE["band"] = _build_band_blocks()
    return _CACHE["nc"], _CACHE["band"]


def kernel(inputs: np.ndarray) -> np.ndarray:
    nc, band = _get_nc()
    x = np.asarray(inputs, dtype=np.float32)
    in_maps = [
        {"x": np.ascontiguousarray(x[c].reshape(1024, 3072)), "band": band}
        for c in range(N_CORES)
    ]
    res = run_bass_kernel_spmd(nc, in_maps, list(range(N_CORES)))
    masks = [res.results[c]["out"] for c in range(N_CORES)]
    return np.stack(masks)[..., None].astype(np.float32)


# revision 24
# speedup vs baseline: 1.0172x; 1.0078x over previous
"""Sauvola binarization kernel for 8 Trainium2 NeuronCores (data-parallel).

Algorithm (per core, one 1024x1024x3 image):
  gray = RGB dot [0.2989, 0.5870, 0.1140]
  m/m2 = 51x51 reflect-padded box means of gray / gray^2 (via two banded
  fp16 matmul passes on the PE: each pass applies the 51-tap reflect box
  along the partition axis and transposes, so H-pass . W-pass returns to
  the original orientation)
  r = 0.5*(max-min) over ALL images' gray, exchanged via an AllGather of
  per-core (-min, max) pairs (~2x cheaper than AllReduce in latency) and
  folded locally.
  thresh = m*(1 + 0.2*(s/r - 1)),  out = (gray > thresh) as f32, computed
  reciprocal-free as  v1 * (1024*2r) > 64*C^2-scaled v2.

Schedule: phase A streams 8 w-chunks (DMA-paced); chunk 7 is processed as
4 row-pieces so the global min/max - and hence the AllGather launch -
clears a couple of us after the final input byte. Phase B (r-independent
t1/t2/s0/qa16/v1/v2) overlaps the collective; only the final per-chunk
mask STT waits for r. Engine placement respects walrus rules: Pool only
runs tensor_scalar / cross-lane max reduces / the collective; every
PSUM-reading elementwise op and all compares live on DVE; copies on Act.
"""
import numpy as np

import concourse.bass as bass
import concourse.mybir as mybir
import concourse.tile as tile
from concourse.bass_utils import run_bass_kernel_spmd

N_CORES = 8
F = mybir.dt.float32
Hh = mybir.dt.float16
W0, W1, W2 = 0.2989, 0.5870, 0.1140
KS = 0.2
HALF = 25
WINDOWS = [(0, 0, 153), (1, 103, 178), (2, 231, 178), (3, 359, 153), (3, 512, 25),
           (4, 487, 25), (4, 512, 153), (5, 615, 178), (6, 743, 178), (7, 871, 153)]
B0_FIRST, B1_FIRST, B0_LAST, B1_LAST = 0, 4, 5, 9
P0PP = (1.0 - KS) / (2601.0 * W0)
C_BASE = 2.0 * KS / (2601.0 ** 2 * W0) * 128.0
# mask compare: v1 * (1024*2r) > s016*qa16 with s016 = sqrt(64*C_BASE^2 * t2)
S0_SCALE = 64.0 * C_BASE * C_BASE
RSUM_SCALE = 1024.0


def _split_multi_waits(nc):
    """walrus here allows one sync wait per instruction; split extras to NOPs."""
    for func in nc.m.functions:
        for bb in func.blocks:
            insts = bb.instructions
            i = 0
            while i < len(insts):
                inst = insts[i]
                si = inst.sync_info
                if si is None or len(si.on_wait) <= 1:
                    i += 1
                    continue
                waits = list(si.on_wait)
                nops = []
                for w in waits[:-1]:
                    nop = mybir.InstNoOp(
                        name=nc.get_next_instruction_name(),
                        sync_info=mybir.SyncInfo(on_wait=[w], on_update=[]),
                        bass_nofuse=True,
                        engine=inst.engine,
                    )
                    nops.append(nop)
                inst.sync_info = mybir.SyncInfo(
                    on_wait=[waits[-1]], on_update=list(si.on_update)
                )
                for k, nop in enumerate(nops):
                    insts.insert(i + k, nop)
                    nc.register_instruction(nop, overwrite=True)
                i += len(nops) + 1


def _build_band_blocks():
    B = np.zeros((1024, 1024), dtype=np.float32)
    idx = np.arange(1024)
    for d in range(-HALF, HALF + 1):
        t = idx + d
        t = np.where(t < 0, -t, t)
        t = np.where(t > 1023, 2046 - t, t)
        np.add.at(B, (idx, t), 1.0)
    # [128 partitions, 10 windows, 178]: one contiguous 3560B descriptor
    # per partition row.
    blocks = np.zeros((128, len(WINDOWS), 178), dtype=np.float16)
    for k, (i, c0, ncols) in enumerate(WINDOWS):
        blocks[:, k, :ncols] = B[c0:c0 + ncols, 128 * i:128 * (i + 1)].T[:, :]
    return blocks


def _emit_matmuls(nc, ps, band_sb, src_of_a, src_of_b):
    """Both banded matmul groups (gray->pa, g2c->pb) for one chunk."""
    pa = ps.tile([128, 1024], F, tag="A")
    pb = ps.tile([128, 1024], F, tag="B")
    for src_of, pt in ((src_of_a, pa), (src_of_b, pb)):
        for k, (i, c0, ncols) in enumerate(WINDOWS):
            nc.tensor.matmul(
                pt[:, c0:c0 + ncols], src_of(i),
                band_sb[:, k, :ncols],
                start=(k in (B0_FIRST, B1_FIRST)),
                stop=(k in (B0_LAST, B1_LAST)))
    return pa, pb


def _build_nc():
    nc = bass.Bass("TRN2", target_bir_lowering=False, debug=False,
                   num_devices=N_CORES)
    x = nc.dram_tensor("x", [1024, 3072], F, kind="ExternalInput")
    band = nc.dram_tensor("band", [128, len(WINDOWS), 178], Hh,
                          kind="ExternalInput")
    out = nc.dram_tensor("out", [1024, 1024], Hh, kind="ExternalOutput")

    AluOp = mybir.AluOpType
    Act = mybir.ActivationFunctionType
    Ax = mybir.AxisListType

    with tile.TileContext(nc) as tc:
        with (
            tc.tile_pool(name="consts", bufs=1) as consts,
            tc.tile_pool(name="xin", bufs=2) as xin,
            tc.tile_pool(name="work", bufs=2) as work,
            tc.tile_pool(name="keep", bufs=1) as keep,
            tc.tile_pool(name="grayp", bufs=3) as grayp,
            tc.tile_pool(name="tkeep", bufs=8) as tkeep,
            tc.tile_pool(name="vkeep", bufs=8) as vkeep,
            tc.tile_pool(name="maskp", bufs=8) as maskp,
            tc.tile_pool(name="ps", bufs=2, space="PSUM") as ps,
            tc.tile_pool(name="dram", bufs=1, space="DRAM") as dram,
        ):
            xc = x.ap().rearrange("(i p) (j w) -> p i j w", p=128, w=384)

            # chunk-0 input DMA first so compute starts ASAP; band second.
            xj0 = xin.tile([128, 8, 384], F, tag="xj")
            nc.sync.dma_start(xj0[:], xc[:, :, 0, :])
            band_sb = consts.tile([128, len(WINDOWS), 178], Hh)
            nc.sync.dma_start(band_sb[:], band.ap())
            bias_sq = consts.tile([128, 1], F)
            nc.gpsimd.memset(bias_sq[:], -25.5)
            bias_t1 = consts.tile([128, 1], F)
            nc.gpsimd.memset(bias_t1[:], -1300.5)

            u2all = keep.tile([128, 8, 8, 128], F)     # gray / W0, all pixels
            accmin = keep.tile([128, 8, 128], Hh)      # fp16 min folds, ch 0-6
            gmaxs = consts.tile([1, 8], F)             # per-chunk max(u2), Pool
            r7 = keep.tile([128, 2, 4], F)             # chunk-7 piece reduces
            ta_tiles, tb_tiles = [], []

            # ---------------- phase A: w-chunks 0..6 ----------------
            prev_copies = None
            for j in range(7):
                xj = xj0 if j == 0 else xin.tile([128, 8, 384], F, tag="xj")
                if j > 0:
                    nc.sync.dma_start(xj[:], xc[:, :, j, :])
                s3 = xj[:].rearrange("p i (w c) -> p i w c", c=3)

                u1 = work.tile([128, 8, 128], F, tag="u1")
                nc.vector.scalar_tensor_tensor(
                    u1[:], s3[:, :, :, 1], W1 / W0, s3[:, :, :, 0],
                    op0=AluOp.mult, op1=AluOp.add)
                u2 = u2all[:, :, j, :]
                nc.vector.scalar_tensor_tensor(
                    u2, s3[:, :, :, 2], W2 / W0, u1[:],
                    op0=AluOp.mult, op1=AluOp.add)

                gray = grayp.tile([128, 8, 128], Hh, tag="gray")
                nc.gpsimd.tensor_scalar(gray[:], u2, W0, None, op0=AluOp.mult)
                g2c = grayp.tile([128, 8, 128], Hh, tag="g2c")
                nc.scalar.activation(g2c[:], gray[:], Act.Square,
                                     bias=bias_sq[:], scale=51.0)

                # max side on Pool (cross-lane max is legal); min folds on DVE
                nc.gpsimd.tensor_reduce(gmaxs[0:1, j:j + 1], u2, Ax.XYZWC,
                                        AluOp.max)
                if j == 0:
                    nc.vector.tensor_copy(accmin[:], gray[:])
                else:
                    nc.vector.tensor_tensor(accmin[:], accmin[:], gray[:],
                                            op=AluOp.min)

                gf, qf = gray, g2c
                pa, pb = _emit_matmuls(nc, ps, band_sb,
                                       lambda i, t=gf: t[:, i, :],
                                       lambda i, t=qf: t[:, i, :])
                # previous chunk's PSUM->fp16 copies go behind this chunk's
                # g2c on Act so g2c[j] never queues behind a PE wait.
                if prev_copies is not None:
                    for src_ps, dst in prev_copies:
                        nc.scalar.copy(dst[:], src_ps[:])
                ta = tkeep.tile([128, 1024], Hh, tag="ta")
                tb = tkeep.tile([128, 1024], Hh, tag="tb")
                ta_tiles.append(ta)
                tb_tiles.append(tb)
                prev_copies = [(pa, ta), (pb, tb)]

            # ---------------- phase A: w-chunk 7 as 4 row-pieces ----------------
            gray7 = keep.tile([128, 8, 128], Hh)
            g2c7 = keep.tile([128, 8, 128], Hh)
            u2ps = []
            for p in range(4):
                xp = xin.tile([128, 2, 384], F, tag="xp", bufs=4)
                nc.sync.dma_start(xp[:], xc[:, 2 * p:2 * p + 2, 7, :])
                s3 = xp[:].rearrange("p i (w c) -> p i w c", c=3)
                u1 = work.tile([128, 2, 128], F, tag="u1p")
                nc.vector.scalar_tensor_tensor(
                    u1[:], s3[:, :, :, 1], W1 / W0, s3[:, :, :, 0],
                    op0=AluOp.mult, op1=AluOp.add)
                u2 = u2all[:, 2 * p:2 * p + 2, 7, :]
                nc.vector.scalar_tensor_tensor(
                    u2, s3[:, :, :, 2], W2 / W0, u1[:],
                    op0=AluOp.mult, op1=AluOp.add)
                u2ps.append(u2)
                gray_s = gray7[:, 2 * p:2 * p + 2, :]
                nc.gpsimd.tensor_scalar(gray_s, u2, W0, None, op0=AluOp.mult)
                nc.scalar.activation(g2c7[:, 2 * p:2 * p + 2, :], gray_s,
                                     Act.Square, bias=bias_sq[:], scale=51.0)
            with tc.high_priority():
                for p in range(4):
                    # lane-only reduces (legal for min) straight off u2 f32
                    nc.vector.tensor_reduce(r7[:, 0, p:p + 1], u2ps[p], Ax.XY,
                                            AluOp.min)
                    nc.vector.tensor_reduce(r7[:, 1, p:p + 1], u2ps[p], Ax.XY,
                                            AluOp.max)

            # ---------------- r-chain + AllGather launch ----------------
            with tc.high_priority():
                # min side: lane-min then negate, cross-lane via max (legal)
                rmin06 = consts.tile([128, 1], F)
                nc.vector.tensor_reduce(
                    rmin06[:], accmin[:].rearrange("p a b -> p (a b)"),
                    Ax.X, AluOp.min)
                n06 = consts.tile([128, 1], F)
                nc.vector.tensor_scalar(n06[:], rmin06[:], -1.0, None,
                                        op0=AluOp.mult)
                r7m = consts.tile([128, 2], F)
                nc.vector.tensor_reduce(r7m[:, 0:1], r7[:, 0, :], Ax.X,
                                        AluOp.min)
                nc.vector.tensor_reduce(r7m[:, 1:2], r7[:, 1, :], Ax.X,
                                        AluOp.max)
                r7s = consts.tile([128, 2], F)
                nc.vector.tensor_scalar(r7s[:, 0:1], r7m[:, 0:1], -W0, None,
                                        op0=AluOp.mult)
                nc.vector.tensor_scalar(r7s[:, 1:2], r7m[:, 1:2], W0, None,
                                        op0=AluOp.mult)
                negall = consts.tile([128, 1], F)
                nc.vector.tensor_tensor(negall[:], n06[:], r7s[:, 0:1],
                                        op=AluOp.max)
                mm1 = consts.tile([1, 2], F)
                nc.gpsimd.tensor_reduce(mm1[:, 0:1], negall[:], Ax.C,
                                        AluOp.max)
                # max side: chunk 0-6 maxima (u2 units) + chunk-7 pieces
                max7 = consts.tile([1, 1], F)
                nc.gpsimd.tensor_reduce(max7[:], r7s[:, 1:2], Ax.C, AluOp.max)
                gmaxu = consts.tile([1, 1], F)
                nc.vector.tensor_reduce(gmaxu[:], gmaxs[:], Ax.X, AluOp.max)
                gmaxg = consts.tile([1, 1], F)
                nc.vector.tensor_scalar(gmaxg[:], gmaxu[:], W0, None,
                                        op0=AluOp.mult)
                nc.vector.tensor_tensor(mm1[:, 1:2], gmaxg[:], max7[:],
                                        op=AluOp.max)

                mm_in = dram.tile([1, 2], F)
                mm_sh = dram.tile([1, 2 * N_CORES], F, addr_space="Shared")
                nc.sync.dma_start(mm_in[:], mm1[:])
                nc.gpsimd.collective_compute(
                    "AllGather", AluOp.bypass,
                    replica_groups=[list(range(N_CORES))],
                    ins=[mm_in.opt()], outs=[mm_sh.opt()])
                mm_b = consts.tile([128, 2 * N_CORES], F)
                nc.sync.dma_start(mm_b[:],
                                  mm_sh[:].to_broadcast((128, 2 * N_CORES)))

            # chunk-7 matmuls + copies
            pa, pb = _emit_matmuls(nc, ps, band_sb,
                                   lambda i: gray7[:, i, :],
                                   lambda i: g2c7[:, i, :])
            ta = tkeep.tile([128, 1024], Hh, tag="ta")
            tb = tkeep.tile([128, 1024], Hh, tag="tb")
            (pa6, ta6), (pb6, tb6) = prev_copies
            nc.scalar.copy(ta6[:], pa6[:])
            nc.vector.tensor_copy(tb6[:], pb6[:])
            nc.scalar.copy(ta[:], pa[:])
            nc.vector.tensor_copy(tb[:], pb[:])
            ta_tiles.append(ta)
            tb_tiles.append(tb)

            # ---------------- phase B (r-independent parts) ----------------
            v1_tiles, v2_tiles = [], []
            for m in range(8):
                qa, qb = _emit_matmuls(
                    nc, ps, band_sb,
                    lambda jj: ta_tiles[jj][:, 128 * m:128 * (m + 1)],
                    lambda jj: tb_tiles[jj][:, 128 * m:128 * (m + 1)])
                qa3 = qa[:].rearrange("p (a b) -> p a b", b=128)
                qb3 = qb[:].rearrange("p (a b) -> p a b", b=128)
                t1 = work.tile([128, 8, 128], F, tag="t1")
                nc.scalar.activation(t1[:], qa3, Act.Square, bias=bias_t1[:],
                                     scale=1.0)
                # fp16 copy of qa so v2 runs as a 2x fp16 TT and qa's psum
                # frees early (v1/t1/qa16 are all immediate post-matmul)
                qa16 = work.tile([128, 8, 128], Hh, tag="qa16")
                nc.scalar.copy(qa16[:], qa3)
                v1 = vkeep.tile([128, 8, 128], Hh, tag="v1")
                nc.vector.scalar_tensor_tensor(
                    v1[:], qa3, -P0PP, u2all[:, m, :, :],
                    op0=AluOp.mult, op1=AluOp.add)
                t2 = work.tile([128, 8, 128], F, tag="t2")
                nc.vector.scalar_tensor_tensor(
                    t2[:], t1[:], -1.0, qb3, op0=AluOp.mult, op1=AluOp.add)
                s016 = work.tile([128, 8, 128], Hh, tag="s016")
                nc.scalar.activation(s016[:], t2[:], Act.Sqrt, scale=S0_SCALE)
                v2 = vkeep.tile([128, 8, 128], Hh, tag="v2")
                nc.vector.tensor_tensor(v2[:], s016[:], qa16[:], op=AluOp.mult)
                v1_tiles.append(v1)
                v2_tiles.append(v2)

            # fence: nothing below may be scheduled before the phase-B ops
            tc.no_sync_barrier()

            # r-dependent chain (tiny, DVE): tree-fold gathered (-min, max),
            # rsum6 = 1024*(gmax - gmin)
            mmv = mm_b[:].rearrange("p (a b) -> p a b", b=2)   # [128, 8, 2]
            f1 = consts.tile([128, 4, 2], F)
            nc.vector.tensor_tensor(f1[:], mmv[:, 0:4, :], mmv[:, 4:8, :],
                                    op=AluOp.max)
            f2 = consts.tile([128, 2, 2], F)
            nc.vector.tensor_tensor(f2[:], f1[:, 0:2, :], f1[:, 2:4, :],
                                    op=AluOp.max)
            f3 = consts.tile([128, 2], F)
            nc.vector.tensor_tensor(f3[:], f2[:, 0, :], f2[:, 1, :],
                                    op=AluOp.max)
            rsum = consts.tile([128, 1], F)
            nc.vector.tensor_tensor(rsum[:], f3[:, 0:1], f3[:, 1:2],
                                    op=AluOp.add)
            rsum6 = consts.tile([128, 1], F)
            nc.vector.tensor_scalar(rsum6[:], rsum[:], RSUM_SCALE, None,
                                    op0=AluOp.mult)

            # ---------------- masks: the only r-dependent sweep ----------------
            out_r = out.ap().rearrange("(m p) (a b) -> m p a b", p=128, b=128)
            for m in range(8):
                mask = maskp.tile([128, 8, 128], Hh, tag="mask")
                nc.vector.scalar_tensor_tensor(
                    mask[:], v1_tiles[m][:], rsum6[:], v2_tiles[m][:],
                    op0=AluOp.mult, op1=AluOp.is_gt)
                nc.sync.dma_start(out_r[m], mask[:])

    _split_multi_waits(nc)
    return nc


_CACHE = {}


def _get_nc():
    if "nc" not in _CACHE:
        _CACHE["nc"] = _build_nc()
        _CACHE["band"] = _build_band_blocks()
    return _CACHE["nc"], _CACHE["band"]


def kernel(inputs: np.ndarray) -> np.ndarray:
    nc, band = _get_nc()
    x = np.asarray(inputs, dtype=np.float32)
    in_maps = [
        {"x": np.ascontiguousarray(x[c].reshape(1024, 3072)), "band": band}
        for c in range(N_CORES)
    ]
    res = run_bass_kernel_spmd(nc, in_maps, list(range(N_CORES)))
    masks = [res.results[c]["out"] for c in range(N_CORES)]
    return np.stack(masks)[..., None].astype(np.float32)


# revision 27
# speedup vs baseline: 1.0281x; 1.0108x over previous
"""Sauvola binarization kernel for 8 Trainium2 NeuronCores (data-parallel).

Algorithm (per core, one 1024x1024x3 image):
  gray = RGB dot [0.2989, 0.5870, 0.1140]
  m/m2 = 51x51 reflect-padded box means of gray / gray^2 (via two banded
  fp16 matmul passes on the PE: each pass applies the 51-tap reflect box
  along the partition axis and transposes, so H-pass . W-pass returns to
  the original orientation)
  r = 0.5*(max-min) over ALL images' gray, exchanged via an AllGather of
  per-core (-min, max) pairs (~2x cheaper than AllReduce in latency) and
  folded locally.
  thresh = m*(1 + 0.2*(s/r - 1)),  out = (gray > thresh) as f32, computed
  reciprocal-free as  v1 * (1024*2r) > 64*C^2-scaled v2.

Schedule: phase A streams 8 w-chunks (DMA-paced); chunk 7 is processed as
4 row-pieces so the global min/max - and hence the AllGather launch -
clears a couple of us after the final input byte. Phase B (r-independent
t1/t2/s0/qa16/v1/v2) overlaps the collective; only the final per-chunk
mask STT waits for r. Engine placement respects walrus rules: Pool only
runs tensor_scalar / cross-lane max reduces / the collective; every
PSUM-reading elementwise op and all compares live on DVE; copies on Act.
"""
import numpy as np

import concourse.bass as bass
import concourse.mybir as mybir
import concourse.tile as tile
from concourse.bass_utils import run_bass_kernel_spmd

N_CORES = 8
F = mybir.dt.float32
Hh = mybir.dt.float16
W0, W1, W2 = 0.2989, 0.5870, 0.1140
KS = 0.2
HALF = 25
WINDOWS = [(0, 0, 153), (1, 103, 178), (2, 231, 178), (3, 359, 153), (3, 512, 25),
           (4, 487, 25), (4, 512, 153), (5, 615, 178), (6, 743, 178), (7, 871, 153)]
B0_FIRST, B1_FIRST, B0_LAST, B1_LAST = 0, 4, 5, 9
P0PP = (1.0 - KS) / (2601.0 * W0)
C_BASE = 2.0 * KS / (2601.0 ** 2 * W0) * 128.0
# mask compare: v1 * (1024*2r) > s016*qa16 with s016 = sqrt(64*C_BASE^2 * t2)
S0_SCALE = 64.0 * C_BASE * C_BASE
RSUM_SCALE = 1024.0


def _split_multi_waits(nc):
    """walrus here allows one sync wait per instruction; split extras to NOPs."""
    for func in nc.m.functions:
        for bb in func.blocks:
            insts = bb.instructions
            i = 0
            while i < len(insts):
                inst = insts[i]
                si = inst.sync_info
                if si is None or len(si.on_wait) <= 1:
                    i += 1
                    continue
                waits = list(si.on_wait)
                nops = []
                for w in waits[:-1]:
                    nop = mybir.InstNoOp(
                        name=nc.get_next_instruction_name(),
                        sync_info=mybir.SyncInfo(on_wait=[w], on_update=[]),
                        bass_nofuse=True,
                        engine=inst.engine,
                    )
                    nops.append(nop)
                inst.sync_info = mybir.SyncInfo(
                    on_wait=[waits[-1]], on_update=list(si.on_update)
                )
                for k, nop in enumerate(nops):
                    insts.insert(i + k, nop)
                    nc.register_instruction(nop, overwrite=True)
                i += len(nops) + 1


def _build_band_blocks():
    B = np.zeros((1024, 1024), dtype=np.float32)
    idx = np.arange(1024)
    for d in range(-HALF, HALF + 1):
        t = idx + d
        t = np.where(t < 0, -t, t)
        t = np.where(t > 1023, 2046 - t, t)
        np.add.at(B, (idx, t), 1.0)
    # [128 partitions, 10 windows, 178]: one contiguous 3560B descriptor
    # per partition row.
    blocks = np.zeros((128, len(WINDOWS), 178), dtype=np.float16)
    for k, (i, c0, ncols) in enumerate(WINDOWS):
        blocks[:, k, :ncols] = B[c0:c0 + ncols, 128 * i:128 * (i + 1)].T[:, :]
    return blocks


def _emit_matmuls(nc, ps, band_sb, src_of_a, src_of_b):
    """Both banded matmul groups (gray->pa, g2c->pb) for one chunk."""
    pa = ps.tile([128, 1024], F, tag="A")
    pb = ps.tile([128, 1024], F, tag="B")
    for src_of, pt in ((src_of_a, pa), (src_of_b, pb)):
        for k, (i, c0, ncols) in enumerate(WINDOWS):
            nc.tensor.matmul(
                pt[:, c0:c0 + ncols], src_of(i),
                band_sb[:, k, :ncols],
                start=(k in (B0_FIRST, B1_FIRST)),
                stop=(k in (B0_LAST, B1_LAST)))
    return pa, pb


def _build_nc():
    nc = bass.Bass("TRN2", target_bir_lowering=False, debug=False,
                   num_devices=N_CORES)
    x = nc.dram_tensor("x", [1024, 3072], F, kind="ExternalInput")
    band = nc.dram_tensor("band", [128, len(WINDOWS), 178], Hh,
                          kind="ExternalInput")
    out = nc.dram_tensor("out", [1024, 1024], Hh, kind="ExternalOutput")

    AluOp = mybir.AluOpType
    Act = mybir.ActivationFunctionType
    Ax = mybir.AxisListType

    with tile.TileContext(nc) as tc:
        with (
            tc.tile_pool(name="consts", bufs=1) as consts,
            tc.tile_pool(name="xin", bufs=2) as xin,
            tc.tile_pool(name="work", bufs=2) as work,
            tc.tile_pool(name="keep", bufs=1) as keep,
            tc.tile_pool(name="grayp", bufs=3) as grayp,
            tc.tile_pool(name="tkeep", bufs=8) as tkeep,
            tc.tile_pool(name="vkeep", bufs=8) as vkeep,
            tc.tile_pool(name="maskp", bufs=8) as maskp,
            tc.tile_pool(name="ps", bufs=2, space="PSUM") as ps,
            tc.tile_pool(name="dram", bufs=1, space="DRAM") as dram,
        ):
            xc = x.ap().rearrange("(i p) (j w) -> p i j w", p=128, w=384)

            # chunk-0 input DMA first so compute starts ASAP; band second.
            xj0 = xin.tile([128, 8, 384], F, tag="xj")
            nc.sync.dma_start(xj0[:], xc[:, :, 0, :])
            band_sb = consts.tile([128, len(WINDOWS), 178], Hh)
            nc.sync.dma_start(band_sb[:], band.ap())
            bias_sq = consts.tile([128, 1], F)
            nc.gpsimd.memset(bias_sq[:], -25.5)
            bias_t1 = consts.tile([128, 1], F)
            nc.gpsimd.memset(bias_t1[:], -1300.5)

            u2all = keep.tile([128, 8, 8, 128], F)     # gray / W0, all pixels
            accmin = keep.tile([128, 8, 128], Hh)      # fp16 min folds, ch 0-6
            gmaxs = consts.tile([1, 8], F)             # per-chunk max(u2), Pool
            r7 = keep.tile([128, 2, 4], F)             # chunk-7 piece reduces
            ta_tiles, tb_tiles = [], []

            # ---------------- phase A: w-chunks 0..6 ----------------
            prev_copies = None
            for j in range(7):
                xj = xj0 if j == 0 else xin.tile([128, 8, 384], F, tag="xj")
                if j > 0:
                    nc.sync.dma_start(xj[:], xc[:, :, j, :])
                s3 = xj[:].rearrange("p i (w c) -> p i w c", c=3)

                u1 = work.tile([128, 8, 128], F, tag="u1")
                nc.vector.scalar_tensor_tensor(
                    u1[:], s3[:, :, :, 1], W1 / W0, s3[:, :, :, 0],
                    op0=AluOp.mult, op1=AluOp.add)
                u2 = u2all[:, :, j, :]
                nc.vector.scalar_tensor_tensor(
                    u2, s3[:, :, :, 2], W2 / W0, u1[:],
                    op0=AluOp.mult, op1=AluOp.add)

                gray = grayp.tile([128, 8, 128], Hh, tag="gray")
                nc.gpsimd.tensor_scalar(gray[:], u2, W0, None, op0=AluOp.mult)
                g2c = grayp.tile([128, 8, 128], Hh, tag="g2c")
                nc.scalar.activation(g2c[:], gray[:], Act.Square,
                                     bias=bias_sq[:], scale=51.0)

                # max side on Pool (cross-lane max is legal); min folds on DVE
                nc.gpsimd.tensor_reduce(gmaxs[0:1, j:j + 1], u2, Ax.XYZWC,
                                        AluOp.max)
                if j == 0:
                    nc.vector.tensor_copy(accmin[:], gray[:])
                else:
                    nc.vector.tensor_tensor(accmin[:], accmin[:], gray[:],
                                            op=AluOp.min)

                gf, qf = gray, g2c
                pa, pb = _emit_matmuls(nc, ps, band_sb,
                                       lambda i, t=gf: t[:, i, :],
                                       lambda i, t=qf: t[:, i, :])
                # previous chunk's PSUM->fp16 copies go behind this chunk's
                # g2c on Act so g2c[j] never queues behind a PE wait.
                if prev_copies is not None:
                    for src_ps, dst in prev_copies:
                        nc.scalar.copy(dst[:], src_ps[:])
                ta = tkeep.tile([128, 1024], Hh, tag="ta")
                tb = tkeep.tile([128, 1024], Hh, tag="tb")
                ta_tiles.append(ta)
                tb_tiles.append(tb)
                prev_copies = [(pa, ta), (pb, tb)]

            # ---------------- phase A: w-chunk 7 as 4 row-pieces ----------------
            gray7 = keep.tile([128, 8, 128], Hh)
            g2c7 = keep.tile([128, 8, 128], Hh)
            u2ps = []
            for p in range(4):
                xp = xin.tile([128, 2, 384], F, tag="xp", bufs=4)
                nc.sync.dma_start(xp[:], xc[:, 2 * p:2 * p + 2, 7, :])
                s3 = xp[:].rearrange("p i (w c) -> p i w c", c=3)
                u1 = work.tile([128, 2, 128], F, tag="u1p")
                nc.vector.scalar_tensor_tensor(
                    u1[:], s3[:, :, :, 1], W1 / W0, s3[:, :, :, 0],
                    op0=AluOp.mult, op1=AluOp.add)
                u2 = u2all[:, 2 * p:2 * p + 2, 7, :]
                nc.vector.scalar_tensor_tensor(
                    u2, s3[:, :, :, 2], W2 / W0, u1[:],
                    op0=AluOp.mult, op1=AluOp.add)
                u2ps.append(u2)
                gray_s = gray7[:, 2 * p:2 * p + 2, :]
                nc.gpsimd.tensor_scalar(gray_s, u2, W0, None, op0=AluOp.mult)
                nc.scalar.activation(g2c7[:, 2 * p:2 * p + 2, :], gray_s,
                                     Act.Square, bias=bias_sq[:], scale=51.0)
            with tc.high_priority():
                for p in range(4):
                    # lane-only reduces (legal for min) straight off u2 f32
                    nc.vector.tensor_reduce(r7[:, 0, p:p + 1], u2ps[p], Ax.XY,
                                            AluOp.min)
                    nc.vector.tensor_reduce(r7[:, 1, p:p + 1], u2ps[p], Ax.XY,
                                            AluOp.max)

            # ---------------- r-chain + AllGather launch ----------------
            with tc.high_priority():
                # min side: lane-min then negate, cross-lane via max (legal)
                rmin06 = consts.tile([128, 1], F)
                nc.vector.tensor_reduce(
                    rmin06[:], accmin[:].rearrange("p a b -> p (a b)"),
                    Ax.X, AluOp.min)
                n06 = consts.tile([128, 1], F)
                nc.vector.tensor_scalar(n06[:], rmin06[:], -1.0, None,
                                        op0=AluOp.mult)
                r7m = consts.tile([128, 2], F)
                nc.vector.tensor_reduce(r7m[:, 0:1], r7[:, 0, :], Ax.X,
                                        AluOp.min)
                nc.vector.tensor_reduce(r7m[:, 1:2], r7[:, 1, :], Ax.X,
                                        AluOp.max)
                r7s = consts.tile([128, 2], F)
                nc.vector.tensor_scalar(r7s[:, 0:1], r7m[:, 0:1], -W0, None,
                                        op0=AluOp.mult)
                nc.vector.tensor_scalar(r7s[:, 1:2], r7m[:, 1:2], W0, None,
                                        op0=AluOp.mult)
                negall = consts.tile([128, 1], F)
                nc.vector.tensor_tensor(negall[:], n06[:], r7s[:, 0:1],
                                        op=AluOp.max)
                mm1 = consts.tile([1, 2], F)
                nc.gpsimd.tensor_reduce(mm1[:, 0:1], negall[:], Ax.C,
                                        AluOp.max)
                # max side: chunk 0-6 maxima (u2 units) + chunk-7 pieces
                max7 = consts.tile([1, 1], F)
                nc.gpsimd.tensor_reduce(max7[:], r7s[:, 1:2], Ax.C, AluOp.max)
                gmaxu = consts.tile([1, 1], F)
                nc.vector.tensor_reduce(gmaxu[:], gmaxs[:], Ax.X, AluOp.max)
                gmaxg = consts.tile([1, 1], F)
                nc.vector.tensor_scalar(gmaxg[:], gmaxu[:], W0, None,
                                        op0=AluOp.mult)
                nc.vector.tensor_tensor(mm1[:, 1:2], gmaxg[:], max7[:],
                                        op=AluOp.max)

                mm_in = dram.tile([1, 2], F)
                mm_sh = dram.tile([1, 2 * N_CORES], F, addr_space="Shared")
                nc.sync.dma_start(mm_in[:], mm1[:])
                nc.gpsimd.collective_compute(
                    "AllGather", AluOp.bypass,
                    replica_groups=[list(range(N_CORES))],
                    ins=[mm_in.opt()], outs=[mm_sh.opt()])
                mm_b = consts.tile([128, 2 * N_CORES], F)
                nc.sync.dma_start(mm_b[:],
                                  mm_sh[:].to_broadcast((128, 2 * N_CORES)))

            # chunk-7 matmuls + copies
            pa, pb = _emit_matmuls(nc, ps, band_sb,
                                   lambda i: gray7[:, i, :],
                                   lambda i: g2c7[:, i, :])
            ta = tkeep.tile([128, 1024], Hh, tag="ta")
            tb = tkeep.tile([128, 1024], Hh, tag="tb")
            (pa6, ta6), (pb6, tb6) = prev_copies
            for lo, hi in ((0, 512), (512, 1024)):
                nc.scalar.copy(ta6[:, lo:hi], pa6[:, lo:hi])
                nc.vector.tensor_copy(tb6[:, lo:hi], pb6[:, lo:hi])
                nc.scalar.copy(ta[:, lo:hi], pa[:, lo:hi])
                nc.vector.tensor_copy(tb[:, lo:hi], pb[:, lo:hi])
            ta_tiles.append(ta)
            tb_tiles.append(tb)

            # ---------------- phase B (r-independent parts) ----------------
            v1_tiles, v2_tiles = [], []
            for m in range(8):
                qa, qb = _emit_matmuls(
                    nc, ps, band_sb,
                    lambda jj: ta_tiles[jj][:, 128 * m:128 * (m + 1)],
                    lambda jj: tb_tiles[jj][:, 128 * m:128 * (m + 1)])
                qa3 = qa[:].rearrange("p (a b) -> p a b", b=128)
                qb3 = qb[:].rearrange("p (a b) -> p a b", b=128)
                t1 = work.tile([128, 8, 128], F, tag="t1")
                nc.scalar.activation(t1[:], qa3, Act.Square, bias=bias_t1[:],
                                     scale=1.0)
                # fp16 copy of qa so v2 runs as a 2x fp16 TT and qa's psum
                # frees early (v1/t1/qa16 are all immediate post-matmul)
                qa16 = work.tile([128, 8, 128], Hh, tag="qa16")
                nc.scalar.copy(qa16[:], qa3)
                v1 = vkeep.tile([128, 8, 128], Hh, tag="v1")
                nc.vector.scalar_tensor_tensor(
                    v1[:], qa3, -P0PP, u2all[:, m, :, :],
                    op0=AluOp.mult, op1=AluOp.add)
                t2 = work.tile([128, 8, 128], F, tag="t2")
                nc.vector.scalar_tensor_tensor(
                    t2[:], t1[:], -1.0, qb3, op0=AluOp.mult, op1=AluOp.add)
                s016 = work.tile([128, 8, 128], Hh, tag="s016")
                nc.scalar.activation(s016[:], t2[:], Act.Sqrt, scale=S0_SCALE)
                v2 = vkeep.tile([128, 8, 128], Hh, tag="v2")
                nc.vector.tensor_tensor(v2[:], s016[:], qa16[:], op=AluOp.mult)
                v1_tiles.append(v1)
                v2_tiles.append(v2)

            # fence: nothing below may be scheduled before the phase-B ops
            tc.no_sync_barrier()

            # r-dependent chain (tiny, DVE): tree-fold gathered (-min, max),
            # rsum6 = 1024*(gmax - gmin)
            mmv = mm_b[:].rearrange("p (a b) -> p a b", b=2)   # [128, 8, 2]
            f1 = consts.tile([128, 4, 2], F)
            nc.vector.tensor_tensor(f1[:], mmv[:, 0:4, :], mmv[:, 4:8, :],
                                    op=AluOp.max)
            f2 = consts.tile([128, 2, 2], F)
            nc.vector.tensor_tensor(f2[:], f1[:, 0:2, :], f1[:, 2:4, :],
                                    op=AluOp.max)
            f3 = consts.tile([128, 2], F)
            nc.vector.tensor_tensor(f3[:], f2[:, 0, :], f2[:, 1, :],
                                    op=AluOp.max)
            rsum = consts.tile([128, 1], F)
            nc.vector.tensor_tensor(rsum[:], f3[:, 0:1], f3[:, 1:2],
                                    op=AluOp.add)
            rsum6 = consts.tile([128, 1], F)
            nc.vector.tensor_scalar(rsum6[:], rsum[:], RSUM_SCALE, None,
                                    op0=AluOp.mult)

            # ---------------- masks: the only r-dependent sweep ----------------
            out_r = out.ap().rearrange("(m p) (a b) -> m p a b", p=128, b=128)
            for m in range(8):
                mask = maskp.tile([128, 8, 128], Hh, tag="mask")
                if m < 6:
                    nc.vector.scalar_tensor_tensor(
                        mask[:], v1_tiles[m][:], rsum6[:], v2_tiles[m][:],
                        op0=AluOp.mult, op1=AluOp.is_gt)
                    nc.sync.dma_start(out_r[m], mask[:])
                else:
                    for lo, hi in ((0, 4), (4, 8)):
                        nc.vector.scalar_tensor_tensor(
                            mask[:, lo:hi, :], v1_tiles[m][:, lo:hi, :],
                            rsum6[:], v2_tiles[m][:, lo:hi, :],
                            op0=AluOp.mult, op1=AluOp.is_gt)
                        nc.sync.dma_start(out_r[m][:, lo:hi, :],
                                          mask[:, lo:hi, :])

    _split_multi_waits(nc)
    return nc


_CACHE = {}


def _get_nc():
    if "nc" not in _CACHE:
        _CACHE["nc"] = _build_nc()
        _CACHE["band"] = _build_band_blocks()
    return _CACHE["nc"], _CACHE["band"]


def kernel(inputs: np.ndarray) -> np.ndarray:
    nc, band = _get_nc()
    x = np.asarray(inputs, dtype=np.float32)
    in_maps = [
        {"x": np.ascontiguousarray(x[c].reshape(1024, 3072)), "band": band}
        for c in range(N_CORES)
    ]
    res = run_bass_kernel_spmd(nc, in_maps, list(range(N_CORES)))
    masks = [res.results[c]["out"] for c in range(N_CORES)]
    return np.stack(masks)[..., None].astype(np.float32)
